# revision 8
# baseline (speedup 1.0000x reference)
"""MultiHead Differential Attention on 8 Trainium2 NeuronCores.

Sharding: data-parallel over batch (B=2), tensor-parallel over heads
(16 heads -> 4 per core).  Core c handles batch c//4, heads (c%4)*4..+4.

Device layout is fully "transposed" (S^T = [k, q] orientation) so that no
on-device transposes are ever needed:
  - projections compute Q^T, K^T directly ([2*Dh, seq]); V naturally [seq, dv]
  - S^T[k,q] = K^T.T @ Q^T  (contraction over d on partitions)
  - softmax row-sums come from an all-ones stationary matmul (M=128 -> the
    sums arrive pre-broadcast across partitions)
  - PV keeps V stationary: O^T[dv,q] accumulates over k-chunks
  - the differential combine d = o1/g - (lam/g)*(s1/s2)*o2 uses a fast DVE
    reciprocal; the leftover s1/g column scale washes out in the RMS norm
    (g = max(|lam|,1) keeps d^2 inside bf16 range)
  - out-proj streams O^T with Wo stationary, producing OUT^T which the host
    transposes and sum-reduces.

The per-qc pipeline is fused (projections for qc, then attention for qc,
then the output projection for qc).  Projection groups compute two outputs
per PSUM tile and drain with one wide copy; out-projection reuses the
o1/o2 PSUM banks (free after the epilogue) so the S-group staging slots
are never blocked behind the epilogue chain.
"""
import numpy as np
import ml_dtypes
from contextlib import ExitStack

import concourse.bass as bass
import concourse.mybir as mybir
import concourse.tile as tile
from concourse import bacc
from concourse.bass_utils import run_bass_kernel_spmd

BF16 = mybir.dt.bfloat16
F32 = mybir.dt.float32
AF = mybir.ActivationFunctionType
ALU = mybir.AluOpType

D_MODEL = 1024
H = 16
DH = 64          # head dim per component
HD = 2 * DH      # 128, per-head width of Q/K/V
N = 2048         # sequence length
B = 2
HPC = 4          # heads per core
LAMBDA_INIT = 0.8
EPS = 1e-5
SCALING = 1.0 / np.sqrt(DH)

MC = D_MODEL // 128   # 8 contraction chunks for projections
QC = 4                # q chunks of 512
KCQ = 4               # k-chunks (128) per q chunk
NKC = 16              # total k chunks

_cache = {}


def _patch_act_tables():
    """Force Exp and Ln to resolve to the single set that contains both,
    so alternating Exp/Ln never reloads activation tables."""
    import concourse.bacc as bacc_mod
    import concourse.hw_specs as hw_specs_mod
    if getattr(bacc_mod, "_act_tables_patched", False):
        return
    orig = hw_specs_mod.get_activation_tables

    def patched(arch):
        t = orig(arch)
        for name, fns in t.items():
            if name != "natural_log_exp_and_others":
                fns.discard(AF.Exp)
                fns.discard(AF.Ln)
        return t

    bacc_mod.get_activation_tables = patched
    bacc_mod._act_tables_patched = True


def _patch_sched_dve_pessimism():
    """Scheduling-only: the Tile scheduler's DVE timing is ~3x optimistic
    versus hardware (measured 402ns for a [128,512] bf16 add vs 133ns
    modeled), which makes it order DVE-dependent matmuls (softmax-sum
    accumulations) too early in the in-order PE stream, stalling the PE.
    Pessimise the modeled DVE clock so the schedule interleaves
    independent matmuls first.  Hardware execution is unaffected."""
    import concourse.hw_specs as hw_specs_mod
    spec = hw_specs_mod.TRN2Spec
    if getattr(spec, "_dve_pessimism", False):
        return
    spec.CYCLE_T = {**spec.CYCLE_T,
                    mybir.EngineType.DVE: 1e9 / 0.40e9}
    spec._dve_pessimism = True


def _build():
    _patch_act_tables()
    _patch_sched_dve_pessimism()
    nc = bacc.Bacc("TRN2", target_bir_lowering=False, debug=False)

    xt_d = nc.dram_tensor("xt", [128, MC, N], BF16, kind="ExternalInput").ap()
    wq_d = nc.dram_tensor("wq", [128, MC, HPC * HD], BF16, kind="ExternalInput").ap()
    wk_d = nc.dram_tensor("wk", [128, MC, HPC * HD], BF16, kind="ExternalInput").ap()
    wv_d = nc.dram_tensor("wv", [128, MC, HPC * HD], BF16, kind="ExternalInput").ap()
    wo_d = nc.dram_tensor("wo", [128, HPC, 8, 128], BF16, kind="ExternalInput").ap()
    lam_d = nc.dram_tensor("lam", [128, 2 * HPC], F32, kind="ExternalInput").ap()
    msk_d = nc.dram_tensor("msk", [128, KCQ, 512], BF16, kind="ExternalInput").ap()
    out_d = nc.dram_tensor("outT", [D_MODEL, N], BF16, kind="ExternalOutput").ap()

    with tile.TileContext(nc) as tc, ExitStack() as ctx:
        # ---- long-lived tiles
        keep = ctx.enter_context(tc.tile_pool(name="keep", bufs=1))
        qt = keep.tile([128, HPC, N], BF16, tag="qt")
        kt = keep.tile([128, HPC, N], BF16, tag="kt")
        vb = keep.tile([128, NKC, 512], BF16, tag="vb")
        otf = [keep.tile([128, N], BF16, tag=f"otf{h}", name=f"otf{h}") for h in range(HPC)]
        lam_t = keep.tile([128, 2 * HPC], F32, tag="lam")
        msk_t = keep.tile([128, KCQ, 512], BF16, tag="msk")
        ones_t = keep.tile([128, 128], BF16, tag="ones")
        eps_t = keep.tile([128, 1], F32, tag="eps")
        wo_t = keep.tile([128, HPC, 8, 128], BF16, tag="wo")

        nc.gpsimd.memset(ones_t[:], 1.0)
        nc.gpsimd.memset(eps_t[:], float(EPS))

        pj = ctx.enter_context(tc.tile_pool(name="proj", bufs=1))
        psum = ctx.enter_context(tc.tile_pool(name="psum", bufs=1, space="PSUM"))
        at = ctx.enter_context(tc.tile_pool(name="att", bufs=2))
        ep = ctx.enter_context(tc.tile_pool(name="esb", bufs=2))
        osb = ctx.enter_context(tc.tile_pool(name="osb", bufs=2))

        xtb = pj.tile([128, MC, N], BF16, tag="xtb")
        wqb = pj.tile([128, MC, HPC * HD], BF16, tag="wqb")
        wkb = pj.tile([128, MC, HPC * HD], BF16, tag="wkb")
        wvb = pj.tile([128, MC, HPC * HD], BF16, tag="wvb")
        # load X^T at per-(mc, seq-chunk) granularity, first seq chunk first,
        # with the weights interleaved: the qc=0 projections become runnable
        # after ~1/4 of the input traffic instead of all of it
        for mc in range(MC):
            nc.sync.dma_start(xtb[:, mc, 0:512], xt_d[:, mc, 0:512])
            nc.sync.dma_start(wvb[:, mc, :], wv_d[:, mc, :])
            nc.sync.dma_start(wqb[:, mc, :], wq_d[:, mc, :])
            nc.sync.dma_start(wkb[:, mc, :], wk_d[:, mc, :])
        nc.sync.dma_start(lam_t[:], lam_d[:])
        nc.sync.dma_start(msk_t[:], msk_d[:])
        nc.sync.dma_start(wo_t[:], wo_d[:])
        for qch in range(1, QC):
            for mc in range(MC):
                nc.sync.dma_start(xtb[:, mc, qch * 512:(qch + 1) * 512],
                                  xt_d[:, mc, qch * 512:(qch + 1) * 512])

        drain_flip = [0]

        def drain(dst, src):
            # alternate PSUM drains between the Act and DVE engines
            drain_flip[0] ^= 1
            if drain_flip[0]:
                nc.scalar.copy(dst, src)
            else:
                nc.vector.tensor_copy(dst, src)

        for qc in range(QC):
            # ============ projections for this q chunk ============
            # each PSUM tile holds two projection outputs; one wide drain
            for scp in range(2):  # V for seq chunks (pairs)
                sc0 = 4 * qc + 2 * scp
                ps = psum.tile([128, 2, 512], F32, tag="sg", name="pjv", bufs=2)
                for i in range(2):
                    for mc in range(MC):
                        nc.tensor.matmul(
                            ps[:, i, :],
                            xtb[:, mc, (sc0 + i) * 128:(sc0 + i + 1) * 128],
                            wvb[:, mc, :],
                            start=(mc == 0), stop=(mc == MC - 1))
                drain(vb[:, sc0:sc0 + 2, :].rearrange("p a b -> p (a b)"),
                      ps[:].rearrange("p a b -> p (a b)"))
            for wsrc, dst in ((wqb, qt), (wkb, kt)):
                for hp in range(2):  # head pairs
                    h0 = 2 * hp
                    ps = psum.tile([128, 2, 512], F32, tag="sg", name="pjqk", bufs=2)
                    for i in range(2):
                        for mc in range(MC):
                            nc.tensor.matmul(
                                ps[:, i, :],
                                wsrc[:, mc, (h0 + i) * HD:(h0 + i + 1) * HD],
                                xtb[:, mc, qc * 512:(qc + 1) * 512],
                                start=(mc == 0), stop=(mc == MC - 1))
                    drain(dst[:, h0:h0 + 2, qc * 512:(qc + 1) * 512],
                          ps[:])

            # ============ attention for this q chunk ============
            for h in range(HPC):
                nkc = KCQ * qc + KCQ  # k chunks in play
                q0 = qc * 512
                s1bc = psum.tile([128, 512], F32, tag="s1bc")
                s2bc = psum.tile([128, 512], F32, tag="s2bc")
                o1 = psum.tile([128, 512], F32, tag="o1", name="o1")
                o2 = psum.tile([128, 512], F32, tag="o2", name="o2")
                ngrp = (nkc + 1) // 2
                hold = [None]
                pending = []

                def emit_b(item, qc=qc, nkc=nkc, ngrp=ngrp, hold=hold,
                           s1bc=s1bc, s2bc=s2bc, o1=o1, o2=o2, h=h):
                    e1, e2, kcs_b, g = item
                    full_pair = (2 * g + 1 < KCQ * qc)
                    if full_pair:
                        ep1 = ep.tile([128, 512], BF16, tag="ep1", name="ep1", bufs=3)
                        nc.vector.tensor_add(ep1[:], e1[:, 0, :], e1[:, 1, :])
                        ep2 = ep.tile([128, 512], BF16, tag="ep2", name="ep2", bufs=3)
                        nc.vector.tensor_add(ep2[:], e2[:, 0, :], e2[:, 1, :])
                        if g % 2 == 0 and 2 * (g + 1) + 1 < KCQ * qc:
                            hold[0] = (ep1, ep2)  # fold into partner pair
                        elif g % 2 == 1 and hold[0] is not None:
                            q1 = ep.tile([128, 512], BF16, tag="q1", name="q1")
                            nc.vector.tensor_add(q1[:], hold[0][0][:], ep1[:])
                            q2 = ep.tile([128, 512], BF16, tag="q2", name="q2")
                            nc.vector.tensor_add(q2[:], hold[0][1][:], ep2[:])
                            hold[0] = None
                            nc.tensor.matmul(s1bc[:], ones_t[:], q1[:],
                                             start=(g == 1), stop=(g == ngrp - 1))
                            nc.tensor.matmul(s2bc[:], ones_t[:], q2[:],
                                             start=(g == 1), stop=(g == ngrp - 1))
                        else:
                            nc.tensor.matmul(s1bc[:], ones_t[:], ep1[:],
                                             start=(g == 0), stop=(g == ngrp - 1))
                            nc.tensor.matmul(s2bc[:], ones_t[:], ep2[:],
                                             start=(g == 0), stop=(g == ngrp - 1))
                    for i, kc in enumerate(kcs_b):
                        j = kc - KCQ * qc
                        w0 = max(0, 128 * j)
                        if j >= 0:  # triangle mask on the diagonal block
                            nc.vector.tensor_mul(
                                e1[:, i, w0:w0 + 128], e1[:, i, w0:w0 + 128],
                                msk_t[:, 0, 0:128])
                            nc.vector.tensor_mul(
                                e2[:, i, w0:w0 + 128], e2[:, i, w0:w0 + 128],
                                msk_t[:, 0, 0:128])
                        st = (kc == 0)
                        sp = (kc == nkc - 1)
                        if not full_pair:
                            nc.tensor.matmul(s1bc[:, w0:512], ones_t[:],
                                             e1[:, i, w0:512],
                                             start=st, stop=sp)
                            nc.tensor.matmul(s2bc[:, w0:512], ones_t[:],
                                             e2[:, i, w0:512],
                                             start=st, stop=sp)
                        nc.tensor.matmul(
                            o1[:, w0:512], vb[:, kc, h * HD:(h + 1) * HD],
                            e1[:, i, w0:512], start=st, stop=sp)
                        nc.tensor.matmul(
                            o2[:, w0:512], vb[:, kc, h * HD:(h + 1) * HD],
                            e2[:, i, w0:512], start=st, stop=sp)

                for g in range(ngrp):
                    kcs = [k for k in (2 * g, 2 * g + 1) if k < nkc]
                    s1g = psum.tile([128, 2, 512], F32, tag="sg", name="s1g", bufs=2)
                    s2g = psum.tile([128, 2, 512], F32, tag="sg", name="s2g", bufs=2)
                    for i, kc in enumerate(kcs):
                        j = kc - KCQ * qc
                        w0 = max(0, 128 * j)  # first valid col of chunk
                        nc.tensor.matmul(
                            s1g[:, i, w0:512], kt[0:64, h, kc * 128:(kc + 1) * 128],
                            qt[0:64, h, q0 + w0:q0 + 512], start=True, stop=True)
                        nc.tensor.matmul(
                            s2g[:, i, w0:512], kt[64:128, h, kc * 128:(kc + 1) * 128],
                            qt[64:128, h, q0 + w0:q0 + 512], start=True, stop=True)
                    e1 = ep.tile([128, 2, 512], BF16, tag="e1", name="e1", bufs=8)
                    e2 = ep.tile([128, 2, 512], BF16, tag="e2", name="e2", bufs=8)
                    j0 = 2 * g - KCQ * qc  # j of first chunk in group
                    if j0 >= 2:
                        for i2, kc2 in enumerate(kcs):
                            w0b = 128 * (kc2 - KCQ * qc)
                            nc.scalar.activation(
                                e1[:, i2, w0b:512], s1g[:, i2, w0b:512],
                                AF.Exp, scale=float(SCALING))
                            nc.scalar.activation(
                                e2[:, i2, w0b:512], s2g[:, i2, w0b:512],
                                AF.Exp, scale=float(SCALING))
                    else:
                        nc.scalar.activation(
                            e1[:].rearrange("p a b -> p (a b)"),
                            s1g[:].rearrange("p a b -> p (a b)"),
                            AF.Exp, scale=float(SCALING))
                        nc.scalar.activation(
                            e2[:].rearrange("p a b -> p (a b)"),
                            s2g[:].rearrange("p a b -> p (a b)"),
                            AF.Exp, scale=float(SCALING))
                    pending.append((e1, e2, kcs, g))
                    if len(pending) > 4:
                        emit_b(pending.pop(0))
                while pending:
                    emit_b(pending.pop(0))
                # ---- epilogue: d = o1/g - (lam/g)*(s1/s2)*o2; the s1/g
                # column scale cancels in the RMS norm.  |lam/g| <= 1 keeps
                # d^2 inside bf16 range.  Division via fast DVE reciprocal.
                r2 = at.tile([128, 512], F32, tag="r2")
                nc.vector.reciprocal_approx_fast(out=r2[:], in_=s2bc[:])
                w = at.tile([128, 512], F32, tag="w")
                nc.vector.scalar_tensor_tensor(
                    w[:], s1bc[:], lam_t[:, h:h + 1], r2[:],
                    ALU.mult, ALU.mult)
                t = at.tile([128, 512], F32, tag="t", bufs=1)
                nc.vector.tensor_mul(t[:], o2[:], w[:])
                d = at.tile([128, 512], BF16, tag="d")
                nc.vector.scalar_tensor_tensor(
                    d[:], o1[:], lam_t[:, HPC + h:HPC + h + 1], t[:],
                    ALU.mult, ALU.subtract)
                osq = at.tile([128, 512], BF16, tag="osq")
                nc.vector.tensor_mul(osq[:], d[:], d[:])
                ssq = psum.tile([128, 512], F32, tag="s1bc", name="ssq")
                nc.tensor.matmul(ssq[:], ones_t[:], osq[:],
                                 start=True, stop=True)
                lnv = at.tile([128, 512], F32, tag="lnv", bufs=1)
                nc.scalar.activation(lnv[:], ssq[:], AF.Ln,
                                     scale=float(1.0 / HD), bias=eps_t[:])
                rr = at.tile([128, 512], BF16, tag="rr")
                nc.scalar.activation(rr[:], lnv[:], AF.Exp, scale=-0.5)
                nc.vector.tensor_mul(otf[h][:, q0:q0 + 512], d[:], rr[:])

            # ============ output projection for this q chunk ============
            # reuses the o1/o2 PSUM banks (free once the epilogue has read
            # them) so S-group staging slots never wait on the epilogue
            for oc in range(8):
                ps = psum.tile([128, 512], F32, tag=("o1" if oc % 2 == 0 else "o2"),
                               name="ops")
                for h in range(HPC):
                    nc.tensor.matmul(
                        ps[:], wo_t[:, h, oc, :],
                        otf[h][:, qc * 512:(qc + 1) * 512],
                        start=(h == 0), stop=(h == HPC - 1))
                ob = osb.tile([128, 512], BF16, tag="ob")
                drain(ob[:], ps[:])
                nc.sync.dma_start(
                    out_d[oc * 128:(oc + 1) * 128, qc * 512:(qc + 1) * 512],
                    ob[:])

    nc.compile()
    return nc


def _prep_inputs(X, Wq, Wk, Wv, Wo, lambda_q1, lambda_k1, lambda_q2,
                 lambda_k2, rms_scale):
    f32 = np.float32
    bf16 = ml_dtypes.bfloat16
    X = np.asarray(X, f32)
    Wq = np.asarray(Wq, f32)
    Wk = np.asarray(Wk, f32)
    Wv = np.asarray(Wv, f32)
    Wo = np.asarray(Wo, f32)
    lam = (np.exp(np.sum(np.asarray(lambda_q1, f32) * np.asarray(lambda_k1, f32), -1))
           - np.exp(np.sum(np.asarray(lambda_q2, f32) * np.asarray(lambda_k2, f32), -1))
           + f32(LAMBDA_INIT)).astype(f32)  # [H]
    # fold rms_scale and (1-lambda_init) into Wo
    wo_f = (Wo.reshape(H, HD, D_MODEL)
            * np.asarray(rms_scale, f32)[None, :, None]
            * f32(1.0 - LAMBDA_INIT)).astype(f32)

    # causal masks for the 4 diagonal-region chunk offsets
    msk = np.zeros((128, KCQ, 512), f32)
    kk = np.arange(128)[:, None]
    cc = np.arange(512)[None, :]
    for j in range(KCQ):
        msk[:, j, :] = (cc >= 128 * j + kk).astype(f32)

    in_maps = []
    for c in range(8):
        b, hg = divmod(c, 4)
        xt = X[b].T.reshape(MC, 128, N).transpose(1, 0, 2)  # [128, MC, N]
        sl = slice(hg * HPC * HD, (hg + 1) * HPC * HD)
        wq = Wq[:, sl].reshape(MC, 128, HPC * HD).transpose(1, 0, 2)
        wk = Wk[:, sl].reshape(MC, 128, HPC * HD).transpose(1, 0, 2)
        wv = Wv[:, sl].reshape(MC, 128, HPC * HD).transpose(1, 0, 2)
        wo = wo_f[hg * HPC:(hg + 1) * HPC].reshape(HPC, HD, 8, 128).transpose(1, 0, 2, 3)
        lv = lam[hg * HPC:(hg + 1) * HPC]
        g = np.maximum(np.abs(lv), f32(1.0)).astype(f32)
        lam_row = np.concatenate([lv / g, 1.0 / g]).astype(f32)
        lam_bc = np.broadcast_to(lam_row[None, :], (128, 2 * HPC))
        in_maps.append({
            "xt": np.ascontiguousarray(xt).astype(bf16),
            "wq": np.ascontiguousarray(wq).astype(bf16),
            "wk": np.ascontiguousarray(wk).astype(bf16),
            "wv": np.ascontiguousarray(wv).astype(bf16),
            "wo": np.ascontiguousarray(wo).astype(bf16),
            "lam": np.ascontiguousarray(lam_bc.astype(f32)),
            "msk": msk.astype(bf16),
        })
    return in_maps


def kernel(X, Wq, Wk, Wv, Wo, lambda_q1, lambda_k1, lambda_q2, lambda_k2,
           rms_scale, _trace=False):
    if "nc" not in _cache:
        _cache["nc"] = _build()
    nc = _cache["nc"]
    in_maps = _prep_inputs(X, Wq, Wk, Wv, Wo, lambda_q1, lambda_k1,
                           lambda_q2, lambda_k2, rms_scale)
    res = run_bass_kernel_spmd(nc, in_maps, list(range(8)), trace=_trace)
    out = np.zeros((B, N, D_MODEL), np.float32)
    for c in range(8):
        b = c // 4
        out[b] += res.results[c]["outT"].T.astype(np.float32)
    _cache["last_exec_ns"] = res.exec_time_ns
    _cache["last_res"] = res
    return out


# revision 9
# speedup vs baseline: 1.1057x; 1.1057x over previous
"""MultiHead Differential Attention on 8 Trainium2 NeuronCores.

Sharding: data-parallel over batch (B=2), tensor-parallel over heads
(16 heads -> 4 per core).  Core c handles batch c//4, heads (c%4)*4..+4.

Device layout is fully "transposed" (S^T = [k, q] orientation) so that no
on-device transposes are ever needed:
  - projections compute Q^T, K^T directly ([2*Dh, seq]); V naturally [seq, dv]
  - S^T[k,q] = K^T.T @ Q^T  (contraction over d on partitions)
  - softmax row-sums come from an all-ones stationary matmul (M=128 -> the
    sums arrive pre-broadcast across partitions)
  - PV keeps V stationary: O^T[dv,q] accumulates over k-chunks
  - the differential combine d = o1/g - (lam/g)*(s1/s2)*o2 uses a fast DVE
    reciprocal; the leftover s1/g column scale washes out in the RMS norm
    (g = max(|lam|,1) keeps d^2 inside bf16 range)
  - out-proj streams O^T with Wo stationary, producing OUT^T which the host
    transposes and sum-reduces.

The per-qc pipeline is fused (projections for qc, then attention for qc,
then the output projection for qc).  Projection groups compute two outputs
per PSUM tile and drain with one wide copy; out-projection reuses the
o1/o2 PSUM banks (free after the epilogue) so the S-group staging slots
are never blocked behind the epilogue chain.
"""
import numpy as np
import ml_dtypes
from contextlib import ExitStack

import concourse.bass as bass
import concourse.mybir as mybir
import concourse.tile as tile
from concourse import bacc
from concourse.bass_utils import run_bass_kernel_spmd

BF16 = mybir.dt.bfloat16
F32 = mybir.dt.float32
AF = mybir.ActivationFunctionType
ALU = mybir.AluOpType

D_MODEL = 1024
H = 16
DH = 64          # head dim per component
HD = 2 * DH      # 128, per-head width of Q/K/V
N = 2048         # sequence length
B = 2
HPC = 4          # heads per core
LAMBDA_INIT = 0.8
EPS = 1e-5
SCALING = 1.0 / np.sqrt(DH)

MC = D_MODEL // 128   # 8 contraction chunks for projections
QC = 4                # q chunks of 512
KCQ = 4               # k-chunks (128) per q chunk
NKC = 16              # total k chunks

_cache = {}


def _patch_act_tables():
    """Force Exp and Ln to resolve to the single set that contains both,
    so alternating Exp/Ln never reloads activation tables."""
    import concourse.bacc as bacc_mod
    import concourse.hw_specs as hw_specs_mod
    if getattr(bacc_mod, "_act_tables_patched", False):
        return
    orig = hw_specs_mod.get_activation_tables

    def patched(arch):
        t = orig(arch)
        for name, fns in t.items():
            if name != "natural_log_exp_and_others":
                fns.discard(AF.Exp)
                fns.discard(AF.Ln)
        return t

    bacc_mod.get_activation_tables = patched
    bacc_mod._act_tables_patched = True


def _patch_sched_dve_pessimism():
    """Scheduling-only: the Tile scheduler's DVE timing is ~3x optimistic
    versus hardware (measured 402ns for a [128,512] bf16 add vs 133ns
    modeled), which makes it order DVE-dependent matmuls (softmax-sum
    accumulations) too early in the in-order PE stream, stalling the PE.
    Pessimise the modeled DVE clock so the schedule interleaves
    independent matmuls first.  Hardware execution is unaffected."""
    import concourse.hw_specs as hw_specs_mod
    spec = hw_specs_mod.TRN2Spec
    if getattr(spec, "_dve_pessimism", False):
        return
    spec.CYCLE_T = {**spec.CYCLE_T,
                    mybir.EngineType.DVE: 1e9 / 0.40e9}
    spec._dve_pessimism = True


def _build():
    _patch_act_tables()
    nc = bacc.Bacc("TRN2", target_bir_lowering=False, debug=False)

    xt_d = nc.dram_tensor("xt", [128, MC, N], BF16, kind="ExternalInput").ap()
    wq_d = nc.dram_tensor("wq", [128, MC, HPC * HD], BF16, kind="ExternalInput").ap()
    wk_d = nc.dram_tensor("wk", [128, MC, HPC * HD], BF16, kind="ExternalInput").ap()
    wv_d = nc.dram_tensor("wv", [128, MC, HPC * HD], BF16, kind="ExternalInput").ap()
    wo_d = nc.dram_tensor("wo", [128, HPC, 8, 128], BF16, kind="ExternalInput").ap()
    lam_d = nc.dram_tensor("lam", [128, 2 * HPC], F32, kind="ExternalInput").ap()
    msk_d = nc.dram_tensor("msk", [128, KCQ, 512], BF16, kind="ExternalInput").ap()
    out_d = nc.dram_tensor("outT", [D_MODEL, N], BF16, kind="ExternalOutput").ap()

    with tile.TileContext(nc) as tc, ExitStack() as ctx:
        # ---- long-lived tiles
        keep = ctx.enter_context(tc.tile_pool(name="keep", bufs=1))
        qt = keep.tile([128, HPC, N], BF16, tag="qt")
        kt = keep.tile([128, HPC, N], BF16, tag="kt")
        vb = keep.tile([128, NKC, 512], BF16, tag="vb")
        otf = [keep.tile([128, N], BF16, tag=f"otf{h}", name=f"otf{h}") for h in range(HPC)]
        lam_t = keep.tile([128, 2 * HPC], F32, tag="lam")
        msk_t = keep.tile([128, KCQ, 512], BF16, tag="msk")
        ones_t = keep.tile([128, 128], BF16, tag="ones")
        eps_t = keep.tile([128, 1], F32, tag="eps")
        wo_t = keep.tile([128, HPC, 8, 128], BF16, tag="wo")

        nc.gpsimd.memset(ones_t[:], 1.0)
        nc.gpsimd.memset(eps_t[:], float(EPS))

        pj = ctx.enter_context(tc.tile_pool(name="proj", bufs=1))
        psum = ctx.enter_context(tc.tile_pool(name="psum", bufs=1, space="PSUM"))
        at = ctx.enter_context(tc.tile_pool(name="att", bufs=2))
        ep = ctx.enter_context(tc.tile_pool(name="esb", bufs=2))
        osb = ctx.enter_context(tc.tile_pool(name="osb", bufs=2))

        xtb = pj.tile([128, MC, N], BF16, tag="xtb")
        wqb = pj.tile([128, MC, HPC * HD], BF16, tag="wqb")
        wkb = pj.tile([128, MC, HPC * HD], BF16, tag="wkb")
        wvb = pj.tile([128, MC, HPC * HD], BF16, tag="wvb")
        # load X^T at per-(mc, seq-chunk) granularity, first seq chunk first,
        # with the weights interleaved: the qc=0 projections become runnable
        # after ~1/4 of the input traffic instead of all of it
        for mc in range(MC):
            nc.sync.dma_start(xtb[:, mc, 0:512], xt_d[:, mc, 0:512])
            nc.sync.dma_start(wvb[:, mc, :], wv_d[:, mc, :])
            nc.sync.dma_start(wqb[:, mc, :], wq_d[:, mc, :])
            nc.sync.dma_start(wkb[:, mc, :], wk_d[:, mc, :])
        nc.sync.dma_start(lam_t[:], lam_d[:])
        nc.sync.dma_start(msk_t[:], msk_d[:])
        nc.sync.dma_start(wo_t[:], wo_d[:])
        for qch in range(1, QC):
            for mc in range(MC):
                nc.sync.dma_start(xtb[:, mc, qch * 512:(qch + 1) * 512],
                                  xt_d[:, mc, qch * 512:(qch + 1) * 512])

        drain_flip = [0]

        def drain(dst, src):
            # alternate PSUM drains between the Act and DVE engines
            drain_flip[0] ^= 1
            if drain_flip[0]:
                nc.scalar.copy(dst, src)
            else:
                nc.vector.tensor_copy(dst, src)

        for qc in range(QC):
            # ============ projections for this q chunk ============
            # each PSUM tile holds two projection outputs; one wide drain
            for scp in range(2):  # V for seq chunks (pairs)
                sc0 = 4 * qc + 2 * scp
                ps = psum.tile([128, 2, 512], F32, tag="sg", name="pjv", bufs=2)
                for i in range(2):
                    for mc in range(MC):
                        nc.tensor.matmul(
                            ps[:, i, :],
                            xtb[:, mc, (sc0 + i) * 128:(sc0 + i + 1) * 128],
                            wvb[:, mc, :],
                            start=(mc == 0), stop=(mc == MC - 1))
                drain(vb[:, sc0:sc0 + 2, :].rearrange("p a b -> p (a b)"),
                      ps[:].rearrange("p a b -> p (a b)"))
            for wsrc, dst in ((wqb, qt), (wkb, kt)):
                for hp in range(2):  # head pairs
                    h0 = 2 * hp
                    ps = psum.tile([128, 2, 512], F32, tag="sg", name="pjqk", bufs=2)
                    for i in range(2):
                        for mc in range(MC):
                            nc.tensor.matmul(
                                ps[:, i, :],
                                wsrc[:, mc, (h0 + i) * HD:(h0 + i + 1) * HD],
                                xtb[:, mc, qc * 512:(qc + 1) * 512],
                                start=(mc == 0), stop=(mc == MC - 1))
                    drain(dst[:, h0:h0 + 2, qc * 512:(qc + 1) * 512],
                          ps[:])

            # ============ attention for this q chunk ============
            for h in range(HPC):
                nkc = KCQ * qc + KCQ  # k chunks in play
                q0 = qc * 512
                s1bc = psum.tile([128, 512], F32, tag="s1bc")
                s2bc = psum.tile([128, 512], F32, tag="s2bc")
                o1 = psum.tile([128, 512], F32, tag="o1", name="o1")
                o2 = psum.tile([128, 512], F32, tag="o2", name="o2")
                ngrp = (nkc + 1) // 2
                hold = [None]
                pending = []

                def emit_b(item, qc=qc, nkc=nkc, ngrp=ngrp, hold=hold,
                           s1bc=s1bc, s2bc=s2bc, o1=o1, o2=o2, h=h):
                    e1, e2, kcs_b, g = item
                    full_pair = (2 * g + 1 < KCQ * qc)
                    if full_pair:
                        ep1 = ep.tile([128, 512], BF16, tag="ep1", name="ep1", bufs=3)
                        nc.vector.tensor_add(ep1[:], e1[:, 0, :], e1[:, 1, :])
                        ep2 = ep.tile([128, 512], BF16, tag="ep2", name="ep2", bufs=3)
                        nc.vector.tensor_add(ep2[:], e2[:, 0, :], e2[:, 1, :])
                        if g % 2 == 0 and 2 * (g + 1) + 1 < KCQ * qc:
                            hold[0] = (ep1, ep2)  # fold into partner pair
                        elif g % 2 == 1 and hold[0] is not None:
                            q1 = ep.tile([128, 512], BF16, tag="q1", name="q1")
                            nc.vector.tensor_add(q1[:], hold[0][0][:], ep1[:])
                            q2 = ep.tile([128, 512], BF16, tag="q2", name="q2")
                            nc.vector.tensor_add(q2[:], hold[0][1][:], ep2[:])
                            hold[0] = None
                            nc.tensor.matmul(s1bc[:], ones_t[:], q1[:],
                                             start=(g == 1), stop=(g == ngrp - 1))
                            nc.tensor.matmul(s2bc[:], ones_t[:], q2[:],
                                             start=(g == 1), stop=(g == ngrp - 1))
                        else:
                            nc.tensor.matmul(s1bc[:], ones_t[:], ep1[:],
                                             start=(g == 0), stop=(g == ngrp - 1))
                            nc.tensor.matmul(s2bc[:], ones_t[:], ep2[:],
                                             start=(g == 0), stop=(g == ngrp - 1))
                    for i, kc in enumerate(kcs_b):
                        j = kc - KCQ * qc
                        w0 = max(0, 128 * j)
                        if j >= 0:  # triangle mask on the diagonal block
                            nc.vector.tensor_mul(
                                e1[:, i, w0:w0 + 128], e1[:, i, w0:w0 + 128],
                                msk_t[:, 0, 0:128])
                            nc.vector.tensor_mul(
                                e2[:, i, w0:w0 + 128], e2[:, i, w0:w0 + 128],
                                msk_t[:, 0, 0:128])
                        st = (kc == 0)
                        sp = (kc == nkc - 1)
                        if not full_pair:
                            nc.tensor.matmul(s1bc[:, w0:512], ones_t[:],
                                             e1[:, i, w0:512],
                                             start=st, stop=sp)
                            nc.tensor.matmul(s2bc[:, w0:512], ones_t[:],
                                             e2[:, i, w0:512],
                                             start=st, stop=sp)
                        nc.tensor.matmul(
                            o1[:, w0:512], vb[:, kc, h * HD:(h + 1) * HD],
                            e1[:, i, w0:512], start=st, stop=sp)
                        nc.tensor.matmul(
                            o2[:, w0:512], vb[:, kc, h * HD:(h + 1) * HD],
                            e2[:, i, w0:512], start=st, stop=sp)

                for g in range(ngrp):
                    kcs = [k for k in (2 * g, 2 * g + 1) if k < nkc]
                    s1g = psum.tile([128, 2, 512], F32, tag="sg", name="s1g", bufs=2)
                    s2g = psum.tile([128, 2, 512], F32, tag="sg", name="s2g", bufs=2)
                    for i, kc in enumerate(kcs):
                        j = kc - KCQ * qc
                        w0 = max(0, 128 * j)  # first valid col of chunk
                        nc.tensor.matmul(
                            s1g[:, i, w0:512], kt[0:64, h, kc * 128:(kc + 1) * 128],
                            qt[0:64, h, q0 + w0:q0 + 512], start=True, stop=True)
                        nc.tensor.matmul(
                            s2g[:, i, w0:512], kt[64:128, h, kc * 128:(kc + 1) * 128],
                            qt[64:128, h, q0 + w0:q0 + 512], start=True, stop=True)
                    e1 = ep.tile([128, 2, 512], BF16, tag="e1", name="e1", bufs=8)
                    e2 = ep.tile([128, 2, 512], BF16, tag="e2", name="e2", bufs=8)
                    j0 = 2 * g - KCQ * qc  # j of first chunk in group
                    if j0 >= 2:
                        for i2, kc2 in enumerate(kcs):
                            w0b = 128 * (kc2 - KCQ * qc)
                            nc.scalar.activation(
                                e1[:, i2, w0b:512], s1g[:, i2, w0b:512],
                                AF.Exp, scale=float(SCALING))
                            nc.scalar.activation(
                                e2[:, i2, w0b:512], s2g[:, i2, w0b:512],
                                AF.Exp, scale=float(SCALING))
                    else:
                        nc.scalar.activation(
                            e1[:].rearrange("p a b -> p (a b)"),
                            s1g[:].rearrange("p a b -> p (a b)"),
                            AF.Exp, scale=float(SCALING))
                        nc.scalar.activation(
                            e2[:].rearrange("p a b -> p (a b)"),
                            s2g[:].rearrange("p a b -> p (a b)"),
                            AF.Exp, scale=float(SCALING))
                    pending.append((e1, e2, kcs, g))
                    if len(pending) > 4:
                        emit_b(pending.pop(0))
                while pending:
                    emit_b(pending.pop(0))
                # ---- epilogue: d = o1/g - (lam/g)*(s1/s2)*o2; the s1/g
                # column scale cancels in the RMS norm.  |lam/g| <= 1 keeps
                # d^2 inside bf16 range.  Division via fast DVE reciprocal.
                r2 = at.tile([128, 512], F32, tag="r2")
                nc.vector.reciprocal_approx_fast(out=r2[:], in_=s2bc[:])
                w = at.tile([128, 512], F32, tag="w")
                nc.vector.scalar_tensor_tensor(
                    w[:], s1bc[:], lam_t[:, h:h + 1], r2[:],
                    ALU.mult, ALU.mult)
                t = at.tile([128, 512], F32, tag="t", bufs=1)
                nc.vector.tensor_mul(t[:], o2[:], w[:])
                d = at.tile([128, 512], BF16, tag="d")
                nc.vector.scalar_tensor_tensor(
                    d[:], o1[:], lam_t[:, HPC + h:HPC + h + 1], t[:],
                    ALU.mult, ALU.subtract)
                osq = at.tile([128, 512], BF16, tag="osq")
                nc.vector.tensor_mul(osq[:], d[:], d[:])
                ssq = psum.tile([128, 512], F32, tag="s1bc", name="ssq")
                nc.tensor.matmul(ssq[:], ones_t[:], osq[:],
                                 start=True, stop=True)
                lnv = at.tile([128, 512], F32, tag="lnv", bufs=1)
                nc.scalar.activation(lnv[:], ssq[:], AF.Ln,
                                     scale=float(1.0 / HD), bias=eps_t[:])
                rr = at.tile([128, 512], BF16, tag="rr")
                nc.scalar.activation(rr[:], lnv[:], AF.Exp, scale=-0.5)
                nc.vector.tensor_mul(otf[h][:, q0:q0 + 512], d[:], rr[:])

            # ============ output projection for this q chunk ============
            # reuses the o1/o2 PSUM banks (free once the epilogue has read
            # them) so S-group staging slots never wait on the epilogue
            for oc in range(8):
                ps = psum.tile([128, 512], F32, tag=("o1" if oc % 2 == 0 else "o2"),
                               name="ops")
                for h in range(HPC):
                    nc.tensor.matmul(
                        ps[:], wo_t[:, h, oc, :],
                        otf[h][:, qc * 512:(qc + 1) * 512],
                        start=(h == 0), stop=(h == HPC - 1))
                ob = osb.tile([128, 512], BF16, tag="ob")
                drain(ob[:], ps[:])
                nc.sync.dma_start(
                    out_d[oc * 128:(oc + 1) * 128, qc * 512:(qc + 1) * 512],
                    ob[:])

    nc.compile()
    return nc


def _prep_inputs(X, Wq, Wk, Wv, Wo, lambda_q1, lambda_k1, lambda_q2,
                 lambda_k2, rms_scale):
    f32 = np.float32
    bf16 = ml_dtypes.bfloat16
    X = np.asarray(X, f32)
    Wq = np.asarray(Wq, f32)
    Wk = np.asarray(Wk, f32)
    Wv = np.asarray(Wv, f32)
    Wo = np.asarray(Wo, f32)
    lam = (np.exp(np.sum(np.asarray(lambda_q1, f32) * np.asarray(lambda_k1, f32), -1))
           - np.exp(np.sum(np.asarray(lambda_q2, f32) * np.asarray(lambda_k2, f32), -1))
           + f32(LAMBDA_INIT)).astype(f32)  # [H]
    # fold rms_scale and (1-lambda_init) into Wo
    wo_f = (Wo.reshape(H, HD, D_MODEL)
            * np.asarray(rms_scale, f32)[None, :, None]
            * f32(1.0 - LAMBDA_INIT)).astype(f32)

    # causal masks for the 4 diagonal-region chunk offsets
    msk = np.zeros((128, KCQ, 512), f32)
    kk = np.arange(128)[:, None]
    cc = np.arange(512)[None, :]
    for j in range(KCQ):
        msk[:, j, :] = (cc >= 128 * j + kk).astype(f32)

    in_maps = []
    for c in range(8):
        b, hg = divmod(c, 4)
        xt = X[b].T.reshape(MC, 128, N).transpose(1, 0, 2)  # [128, MC, N]
        sl = slice(hg * HPC * HD, (hg + 1) * HPC * HD)
        wq = Wq[:, sl].reshape(MC, 128, HPC * HD).transpose(1, 0, 2)
        wk = Wk[:, sl].reshape(MC, 128, HPC * HD).transpose(1, 0, 2)
        wv = Wv[:, sl].reshape(MC, 128, HPC * HD).transpose(1, 0, 2)
        wo = wo_f[hg * HPC:(hg + 1) * HPC].reshape(HPC, HD, 8, 128).transpose(1, 0, 2, 3)
        lv = lam[hg * HPC:(hg + 1) * HPC]
        g = np.maximum(np.abs(lv), f32(1.0)).astype(f32)
        lam_row = np.concatenate([lv / g, 1.0 / g]).astype(f32)
        lam_bc = np.broadcast_to(lam_row[None, :], (128, 2 * HPC))
        in_maps.append({
            "xt": np.ascontiguousarray(xt).astype(bf16),
            "wq": np.ascontiguousarray(wq).astype(bf16),
            "wk": np.ascontiguousarray(wk).astype(bf16),
            "wv": np.ascontiguousarray(wv).astype(bf16),
            "wo": np.ascontiguousarray(wo).astype(bf16),
            "lam": np.ascontiguousarray(lam_bc.astype(f32)),
            "msk": msk.astype(bf16),
        })
    return in_maps


def kernel(X, Wq, Wk, Wv, Wo, lambda_q1, lambda_k1, lambda_q2, lambda_k2,
           rms_scale, _trace=False):
    if "nc" not in _cache:
        _cache["nc"] = _build()
    nc = _cache["nc"]
    in_maps = _prep_inputs(X, Wq, Wk, Wv, Wo, lambda_q1, lambda_k1,
                           lambda_q2, lambda_k2, rms_scale)
    res = run_bass_kernel_spmd(nc, in_maps, list(range(8)), trace=_trace)
    out = np.zeros((B, N, D_MODEL), np.float32)
    for c in range(8):
        b = c // 4
        out[b] += res.results[c]["outT"].T.astype(np.float32)
    _cache["last_exec_ns"] = res.exec_time_ns
    _cache["last_res"] = res
    return out
